# revision 6
# baseline (speedup 1.0000x reference)
"""Low-pass FFT filtering kernel for Trainium2 (8 NeuronCores), v2.

Math: per (batch b, channel i), X = x[b,:,:,i] (256x256):
    out_i = P @ X_i + X_i @ P,  P = W @ W.T,  W [256, 31] orthonormal
    {1/sqrt(n), sqrt(2/n)cos(2pi k t/n), -sqrt(2/n)sin(2pi k t/n)}_{k=1..15}.

v2 vs v1: the host sends ONE fp16 copy of x per core (channel-major
[m, (i, n)]); the transposed layout needed for the X_i @ P term is
produced on-device by the DMA XBAR transpose (SBUF->SBUF, no HBM
traffic), cutting HBM bytes/core from ~13.7MB to ~8.8MB. C (=W^T X) and
D (=(X W)^T) land in one merged SBUF tile LR [126, cw]:
  partitions 0..30  Wt const | 31 zero | 32..62 D (63: 0)  ("L" = lhsT)
  partitions 64..94 C        | 95 zero | 96..126 Wt         ("R" = rhs)
so one [64, 512] ACT copy moves C and D together, and the phase-2
matmul is a single fused K=63 pass per (channel, m-half):
  p2 = W @ C_i + D_ij^T @ W^T  (both terms in one PSUM accumulation).
Constants are loaded once into a ring of 3 LR buffers instead of
per-chunk. Outputs ride the SP ring tail + SWDGE; transposes + consts
ride the ACT ring. Sharding: batch b -> core b (no communication).
"""

import os
import sys
import types

import numpy as np

import concourse.bass as bass
import concourse.bacc as bacc
import concourse.tile as tile
from concourse import mybir
from concourse.bass_utils import run_bass_kernel_spmd

B, M, N, I = 8, 256, 256, 32
KMAX = 16           # modes kept: 0..15
R = 2 * KMAX - 1    # 31 real basis vectors
FREE = I * N        # 8192
NCH = 8             # chunks
CW = FREE // NCH    # 1024 cols per chunk = 4 channels
CHC = I // NCH      # channels per chunk
NLR = 3             # LR buffer ring depth
F32 = mybir.dt.float32
F16 = mybir.dt.float16
NPDT = np.float16

LAST_RESULTS = None  # BassKernelResults of the most recent run (for test.py)


def _ensure_ntff_hook():
    """Provide antenv.axon_hooks if the image lacks it, so trace=True works."""
    try:
        from antenv.axon_hooks import get_axon_ntff_profile_hook  # noqa: F401
        return
    except ImportError:
        pass
    try:
        from trn_agent_boot.trn_boot import _ntff_profile_via_ctypes
        hook = _ntff_profile_via_ctypes("/opt/axon/libaxon_pjrt.so")
    except Exception:
        hook = None
    mod = types.ModuleType("antenv.axon_hooks")
    _state = {"hook": hook}
    mod.get_axon_ntff_profile_hook = lambda: _state["hook"]
    mod.set_axon_ntff_profile_hook = lambda h: _state.update(hook=h)
    sys.modules["antenv.axon_hooks"] = mod
    try:
        import antenv
        antenv.axon_hooks = mod
    except ImportError:
        pass


def _basis():
    t = np.arange(N)
    cols = [np.ones(N) / np.sqrt(N)]
    for k in range(1, KMAX):
        cols.append(np.sqrt(2.0 / N) * np.cos(2 * np.pi * k * t / N))
        cols.append(-np.sqrt(2.0 / N) * np.sin(2 * np.pi * k * t / N))
    return np.stack(cols, axis=1).astype(np.float32)  # [256, 31]


def _build_nc():
    nc = bacc.Bacc("TRN2", target_bir_lowering=False, debug=False,
                   enable_asserts=False, num_devices=8)

    xc = nc.declare_dram_parameter("xc", [M, FREE], F16, isOutput=False)
    w2 = nc.declare_dram_parameter("w2", [128, 64], F16, isOutput=False)
    wzl = nc.declare_dram_parameter("wzl", [32, CW], F16, isOutput=False)
    wzr = nc.declare_dram_parameter("wzr", [31, CW], F16, isOutput=False)
    out = nc.declare_dram_parameter("out", [M, FREE], F16, isOutput=True)

    with tile.TileContext(nc) as tc:
        with (
            tc.tile_pool(name="const", bufs=1) as constp,
            tc.tile_pool(name="xin", bufs=3) as xinp,
            tc.tile_pool(name="tt", bufs=3) as ttp,
            tc.tile_pool(name="oput", bufs=3) as outp,
            tc.tile_pool(name="pcd", bufs=4, space=bass.MemorySpace.PSUM) as pcdp,
            tc.tile_pool(name="p2", bufs=2, space=bass.MemorySpace.PSUM) as p2p,
        ):
            w2sb = constp.tile([128, 64], F16)
            nc.scalar.dma_start(out=w2sb[:], in_=w2[:])
            # rings of L/R buffers; const rows filled once per buffer
            lgs, rgs = [], []
            for r_ in range(NLR):
                lg_t = constp.tile([64, CW], F16, name=f"lg{r_}")
                nc.scalar.dma_start(out=lg_t[0:32, :], in_=wzl[:])
                lgs.append(lg_t)
                rg_t = constp.tile([63, CW], F16, name=f"rg{r_}")
                nc.scalar.dma_start(out=rg_t[32:63, :], in_=wzr[:])
                rgs.append(rg_t)

            outs_todo = []
            for g in range(NCH):
                gsl = slice(g * CW, (g + 1) * CW)
                Lg = lgs[g % NLR]
                Rg = rgs[g % NLR]

                # one DMA per chunk: both m-halves into [128, 2, CW]
                xin = xinp.tile([128, 2, CW], F16, tag="xin")
                nc.sync.dma_start(
                    out=xin[:],
                    in_=xc[:, gsl].rearrange("(h p) c -> p h c", h=2))

                # XBAR transpose (SBUF->SBUF) of the whole chunk:
                # tt[n', gg, q] = xin[q, gg//8, (gg%8)*128 + n']
                # where col (h*1024 + il*256 + n) -> gg = h*8 + 2*il + n//128
                tt = ttp.tile([128, 2 * CW // 128, 128], F16, tag="tt")
                nc.scalar.dma_start(out=tt[:], in_=xin[:], transpose=True)
                # view [128, kh, x, 128] with gg = x*2 + kh, x = h*4 + il
                tt_r = tt[:].rearrange("p (x y) m -> p y x m", y=2)

                for f in range(2):  # 512-col f-block = 2 channels
                    fsl = slice(f * 512, (f + 1) * 512)
                    pcd = pcdp.tile([64, 512], F32, tag="pcd")
                    # C = W^T X into rows 0..30 (31: zero col of w2)
                    nc.tensor.matmul(pcd[0:32, :], w2sb[:, 0:32],
                                     xin[:, 0, fsl], start=True, stop=False)
                    nc.tensor.matmul(pcd[0:32, :], w2sb[:, 32:64],
                                     xin[:, 1, fsl], start=False, stop=True)
                    # D = (X W)^T into rows 32..62 (63: zero); cols (il, h, m')
                    dv = pcd[32:64, :].rearrange("p (i m) -> p i m", i=2)
                    for h in range(2):
                        dout = dv[:, :, h * 128:(h + 1) * 128]
                        for kh in range(2):
                            x0 = h * 4 + 2 * f
                            nc.tensor.matmul(
                                dout, w2sb[:, kh * 32:(kh + 1) * 32],
                                tt_r[:, kh, x0:x0 + 2, :],
                                start=(kh == 0), stop=(kh == 1))
                    # copies: D -> Lg rows 32..63 (ACT), C -> Rg 0..31 (DVE)
                    nc.scalar.copy(Lg[32:64, fsl], pcd[32:64, :])
                    nc.vector.tensor_copy(Rg[0:32, fsl], pcd[0:32, :])

                oo = outp.tile([128, 2, CW], F16, tag="oo")
                for j in range(2):  # m-half of the output
                    p2 = p2p.tile([128, CW], F32, tag="p2")
                    for ip in range(CHC):
                        c0 = ip * 256 + j * 128
                        nc.tensor.matmul(p2[:, ip * 256:(ip + 1) * 256],
                                         Lg[0:63, c0:c0 + 128],
                                         Rg[0:63, ip * 256:(ip + 1) * 256],
                                         start=True, stop=True)
                    nc.vector.tensor_copy(oo[:, j, :], p2[:])

                dst = out[:, gsl].rearrange("(j p) c -> p j c", j=2)
                if g >= NCH - 2:
                    # last chunks: SWDGE so they don't queue behind the SP tail
                    nc.gpsimd.dma_start(out=dst, in_=oo[:])
                else:
                    outs_todo.append((dst, oo))

            # SP-ring outputs at the tail: FIFO-after all input DMAs
            for dst, oo in outs_todo:
                nc.sync.dma_start(out=dst, in_=oo[:])

    nc.finalize()
    return nc


_NC = None


def kernel(x: np.ndarray) -> np.ndarray:
    global _NC, LAST_RESULTS
    x = np.asarray(x)
    assert x.shape == (B, M, N, I), x.shape

    W = _basis().astype(NPDT)          # [256, 31]
    Wt = W.T.copy()                    # [31, 256]
    zcol = np.zeros((128, 1), NPDT)
    w2_np = np.concatenate([W[0:128, :], zcol, W[128:256, :], zcol],
                           axis=1)                                # [128, 64]
    wtile = np.tile(Wt, (1, CHC))                                 # [31, CW]
    zrow = np.zeros((1, CW), NPDT)
    wzl_np = np.concatenate([wtile, zrow], axis=0)                # [32, CW]
    wzr_np = wtile                                                # [31, CW]

    if _NC is None:
        _NC = _build_nc()

    xq = np.asarray(x, dtype=NPDT)
    in_maps = []
    for b in range(B):
        xcm = np.ascontiguousarray(xq[b].transpose(0, 2, 1)).reshape(M, FREE)
        in_maps.append({
            "xc": xcm, "w2": w2_np, "wzl": wzl_np, "wzr": wzr_np,
        })

    trace = bool(int(os.environ.get("KERNEL_TRACE", "0")))
    if trace:
        _ensure_ntff_hook()
    last_err = None
    for attempt in range(3):
        try:
            LAST_RESULTS = run_bass_kernel_spmd(_NC, in_maps, list(range(B)),
                                                trace=trace and attempt == 0)
            break
        except Exception as e:  # rare transient NRT_EXEC_UNIT_UNRECOVERABLE
            last_err = e
            import time as _time
            _time.sleep(2.0)
            try:
                import jax
                jax.clear_caches()
                jax.extend.backend.clear_backends()
            except Exception:
                pass
    else:
        raise last_err

    out = np.empty((B, M, N, I), np.float32)
    for b in range(B):
        dev = LAST_RESULTS.results[b]["out"].astype(np.float32).reshape(M, I, N)
        out[b] = dev.transpose(0, 2, 1)
    return out


# revision 7
# speedup vs baseline: 1.0460x; 1.0460x over previous
"""Low-pass FFT filtering kernel for Trainium2 (8 NeuronCores), v2.

Math: per (batch b, channel i), X = x[b,:,:,i] (256x256):
    out_i = P @ X_i + X_i @ P,  P = W @ W.T,  W [256, 31] orthonormal
    {1/sqrt(n), sqrt(2/n)cos(2pi k t/n), -sqrt(2/n)sin(2pi k t/n)}_{k=1..15}.

v2 vs v1: the host sends ONE fp16 copy of x per core (channel-major
[m, (i, n)]); the transposed layout needed for the X_i @ P term is
produced on-device by the DMA XBAR transpose (SBUF->SBUF, no HBM
traffic), cutting HBM bytes/core from ~13.7MB to ~8.8MB. C (=W^T X) and
D (=(X W)^T) land in one merged SBUF tile LR [126, cw]:
  partitions 0..30  Wt const | 31 zero | 32..62 D (63: 0)  ("L" = lhsT)
  partitions 64..94 C        | 95 zero | 96..126 Wt         ("R" = rhs)
so one [64, 512] ACT copy moves C and D together, and the phase-2
matmul is a single fused K=63 pass per (channel, m-half):
  p2 = W @ C_i + D_ij^T @ W^T  (both terms in one PSUM accumulation).
Constants are loaded once into a ring of 3 LR buffers instead of
per-chunk. Outputs ride the SP ring tail + SWDGE; transposes + consts
ride the ACT ring. Sharding: batch b -> core b (no communication).
"""

import os
import sys
import types

import numpy as np

import concourse.bass as bass
import concourse.bacc as bacc
import concourse.tile as tile
from concourse import mybir
from concourse.bass_utils import run_bass_kernel_spmd

B, M, N, I = 8, 256, 256, 32
KMAX = 16           # modes kept: 0..15
R = 2 * KMAX - 1    # 31 real basis vectors
FREE = I * N        # 8192
NCH = 8             # chunks
CW = FREE // NCH    # 1024 cols per chunk = 4 channels
CHC = I // NCH      # channels per chunk
NLR = 3             # LR buffer ring depth
F32 = mybir.dt.float32
F16 = mybir.dt.float16
NPDT = np.float16

LAST_RESULTS = None  # BassKernelResults of the most recent run (for test.py)


def _ensure_ntff_hook():
    """Provide antenv.axon_hooks if the image lacks it, so trace=True works."""
    try:
        from antenv.axon_hooks import get_axon_ntff_profile_hook  # noqa: F401
        return
    except ImportError:
        pass
    try:
        from trn_agent_boot.trn_boot import _ntff_profile_via_ctypes
        hook = _ntff_profile_via_ctypes("/opt/axon/libaxon_pjrt.so")
    except Exception:
        hook = None
    mod = types.ModuleType("antenv.axon_hooks")
    _state = {"hook": hook}
    mod.get_axon_ntff_profile_hook = lambda: _state["hook"]
    mod.set_axon_ntff_profile_hook = lambda h: _state.update(hook=h)
    sys.modules["antenv.axon_hooks"] = mod
    try:
        import antenv
        antenv.axon_hooks = mod
    except ImportError:
        pass


def _basis():
    t = np.arange(N)
    cols = [np.ones(N) / np.sqrt(N)]
    for k in range(1, KMAX):
        cols.append(np.sqrt(2.0 / N) * np.cos(2 * np.pi * k * t / N))
        cols.append(-np.sqrt(2.0 / N) * np.sin(2 * np.pi * k * t / N))
    return np.stack(cols, axis=1).astype(np.float32)  # [256, 31]


def _build_nc():
    nc = bacc.Bacc("TRN2", target_bir_lowering=False, debug=False,
                   enable_asserts=False, num_devices=8)

    xc = nc.declare_dram_parameter("xc", [M, FREE], F16, isOutput=False)
    w2 = nc.declare_dram_parameter("w2", [128, 64], F16, isOutput=False)
    wzl = nc.declare_dram_parameter("wzl", [32, CW], F16, isOutput=False)
    wzr = nc.declare_dram_parameter("wzr", [31, CW], F16, isOutput=False)
    out = nc.declare_dram_parameter("out", [M, FREE], F16, isOutput=True)

    with tile.TileContext(nc) as tc:
        with (
            tc.tile_pool(name="const", bufs=1) as constp,
            tc.tile_pool(name="xin", bufs=4) as xinp,
            tc.tile_pool(name="tt", bufs=3) as ttp,
            tc.tile_pool(name="oput", bufs=3) as outp,
            tc.tile_pool(name="pcd", bufs=4, space=bass.MemorySpace.PSUM) as pcdp,
            tc.tile_pool(name="p2", bufs=2, space=bass.MemorySpace.PSUM) as p2p,
        ):
            # consts ride SWDGE so the ACT ring is free for transposes
            w2sb = constp.tile([128, 64], F16)
            nc.gpsimd.dma_start(out=w2sb[:], in_=w2[:])
            # rings of L/R buffers; const rows filled once per buffer
            lgs, rgs = [], []
            for r_ in range(NLR):
                lg_t = constp.tile([64, CW], F16, name=f"lg{r_}")
                nc.gpsimd.dma_start(out=lg_t[0:32, :], in_=wzl[:])
                lgs.append(lg_t)
                rg_t = constp.tile([63, CW], F16, name=f"rg{r_}")
                nc.gpsimd.dma_start(out=rg_t[32:63, :], in_=wzr[:])
                rgs.append(rg_t)

            outs_todo = []
            for g in range(NCH):
                gsl = slice(g * CW, (g + 1) * CW)
                Lg = lgs[g % NLR]
                Rg = rgs[g % NLR]

                # one DMA per chunk: both m-halves into [128, 2, CW]
                xin = xinp.tile([128, 2, CW], F16, tag="xin")
                nc.sync.dma_start(
                    out=xin[:],
                    in_=xc[:, gsl].rearrange("(h p) c -> p h c", h=2))

                # XBAR transpose (SBUF->SBUF) of the whole chunk:
                # tt[n', gg, q] = xin[q, gg//8, (gg%8)*128 + n']
                # where col (h*1024 + il*256 + n) -> gg = h*8 + 2*il + n//128
                tt = ttp.tile([128, 2 * CW // 128, 128], F16, tag="tt")
                nc.scalar.dma_start(out=tt[:], in_=xin[:], transpose=True)
                # view [128, kh, x, 128] with gg = x*2 + kh, x = h*4 + il
                tt_r = tt[:].rearrange("p (x y) m -> p y x m", y=2)

                for f in range(2):  # 512-col f-block = 2 channels
                    fsl = slice(f * 512, (f + 1) * 512)
                    pcd = pcdp.tile([64, 512], F32, tag="pcd")
                    # C = W^T X into rows 0..30 (31: zero col of w2)
                    nc.tensor.matmul(pcd[0:32, :], w2sb[:, 0:32],
                                     xin[:, 0, fsl], start=True, stop=False)
                    nc.tensor.matmul(pcd[0:32, :], w2sb[:, 32:64],
                                     xin[:, 1, fsl], start=False, stop=True)
                    # D = (X W)^T into rows 32..62 (63: zero); cols (il, h, m')
                    dv = pcd[32:64, :].rearrange("p (i m) -> p i m", i=2)
                    for h in range(2):
                        dout = dv[:, :, h * 128:(h + 1) * 128]
                        for kh in range(2):
                            x0 = h * 4 + 2 * f
                            nc.tensor.matmul(
                                dout, w2sb[:, kh * 32:(kh + 1) * 32],
                                tt_r[:, kh, x0:x0 + 2, :],
                                start=(kh == 0), stop=(kh == 1))
                    # copies: D -> Lg rows 32..63 (ACT), C -> Rg 0..31 (DVE)
                    nc.scalar.copy(Lg[32:64, fsl], pcd[32:64, :])
                    nc.vector.tensor_copy(Rg[0:32, fsl], pcd[0:32, :])

                oo = outp.tile([128, 2, CW], F16, tag="oo")
                for j in range(2):  # m-half of the output
                    p2 = p2p.tile([128, CW], F32, tag="p2")
                    for ip in range(CHC):
                        c0 = ip * 256 + j * 128
                        nc.tensor.matmul(p2[:, ip * 256:(ip + 1) * 256],
                                         Lg[0:63, c0:c0 + 128],
                                         Rg[0:63, ip * 256:(ip + 1) * 256],
                                         start=True, stop=True)
                    nc.vector.tensor_copy(oo[:, j, :], p2[:])

                dst = out[:, gsl].rearrange("(j p) c -> p j c", j=2)
                if g >= NCH - 2:
                    # last chunks: SWDGE so they don't queue behind the SP tail
                    nc.gpsimd.dma_start(out=dst, in_=oo[:])
                elif g >= NCH - 4:
                    # ACT ring is free once the transposes drain
                    nc.scalar.dma_start(out=dst, in_=oo[:])
                else:
                    outs_todo.append((dst, oo))

            # SP-ring outputs at the tail: FIFO-after all input DMAs
            for dst, oo in outs_todo:
                nc.sync.dma_start(out=dst, in_=oo[:])

    nc.finalize()
    return nc


_NC = None


def kernel(x: np.ndarray) -> np.ndarray:
    global _NC, LAST_RESULTS
    x = np.asarray(x)
    assert x.shape == (B, M, N, I), x.shape

    W = _basis().astype(NPDT)          # [256, 31]
    Wt = W.T.copy()                    # [31, 256]
    zcol = np.zeros((128, 1), NPDT)
    w2_np = np.concatenate([W[0:128, :], zcol, W[128:256, :], zcol],
                           axis=1)                                # [128, 64]
    wtile = np.tile(Wt, (1, CHC))                                 # [31, CW]
    zrow = np.zeros((1, CW), NPDT)
    wzl_np = np.concatenate([wtile, zrow], axis=0)                # [32, CW]
    wzr_np = wtile                                                # [31, CW]

    if _NC is None:
        _NC = _build_nc()

    xq = np.asarray(x, dtype=NPDT)
    in_maps = []
    for b in range(B):
        xcm = np.ascontiguousarray(xq[b].transpose(0, 2, 1)).reshape(M, FREE)
        in_maps.append({
            "xc": xcm, "w2": w2_np, "wzl": wzl_np, "wzr": wzr_np,
        })

    trace = bool(int(os.environ.get("KERNEL_TRACE", "0")))
    if trace:
        _ensure_ntff_hook()
    last_err = None
    for attempt in range(3):
        try:
            LAST_RESULTS = run_bass_kernel_spmd(_NC, in_maps, list(range(B)),
                                                trace=trace and attempt == 0)
            break
        except Exception as e:  # rare transient NRT_EXEC_UNIT_UNRECOVERABLE
            last_err = e
            import time as _time
            _time.sleep(2.0)
            try:
                import jax
                jax.clear_caches()
                jax.extend.backend.clear_backends()
            except Exception:
                pass
    else:
        raise last_err

    out = np.empty((B, M, N, I), np.float32)
    for b in range(B):
        dev = LAST_RESULTS.results[b]["out"].astype(np.float32).reshape(M, I, N)
        out[b] = dev.transpose(0, 2, 1)
    return out


# revision 11
# speedup vs baseline: 1.0894x; 1.0415x over previous
"""Low-pass FFT filtering kernel for Trainium2 (8 NeuronCores), v2.

Math: per (batch b, channel i), X = x[b,:,:,i] (256x256):
    out_i = P @ X_i + X_i @ P,  P = W @ W.T,  W [256, 31] orthonormal
    {1/sqrt(n), sqrt(2/n)cos(2pi k t/n), -sqrt(2/n)sin(2pi k t/n)}_{k=1..15}.

v2 vs v1: the host sends ONE fp16 copy of x per core (channel-major
[m, (i, n)]); the transposed layout needed for the X_i @ P term is
produced on-device by the DMA XBAR transpose (SBUF->SBUF, no HBM
traffic), cutting HBM bytes/core from ~13.7MB to ~8.8MB. C (=W^T X) and
D (=(X W)^T) land in one merged SBUF tile LR [126, cw]:
  partitions 0..30  Wt const | 31 zero | 32..62 D (63: 0)  ("L" = lhsT)
  partitions 64..94 C        | 95 zero | 96..126 Wt         ("R" = rhs)
so one [64, 512] ACT copy moves C and D together, and the phase-2
matmul is a single fused K=63 pass per (channel, m-half):
  p2 = W @ C_i + D_ij^T @ W^T  (both terms in one PSUM accumulation).
Constants are loaded once into a ring of 3 LR buffers instead of
per-chunk. Outputs ride the SP ring tail + SWDGE; transposes + consts
ride the ACT ring. Sharding: batch b -> core b (no communication).
"""

import os
import sys
import types

import numpy as np

import concourse.bass as bass
import concourse.bacc as bacc
import concourse.tile as tile
from concourse import mybir
from concourse.bass_utils import run_bass_kernel_spmd

B, M, N, I = 8, 256, 256, 32
KMAX = 16           # modes kept: 0..15
R = 2 * KMAX - 1    # 31 real basis vectors
FREE = I * N        # 8192
NCH = 8             # chunks
CW = FREE // NCH    # 1024 cols per chunk = 4 channels
CHC = I // NCH      # channels per chunk
NLR = 3             # LR buffer ring depth
F32 = mybir.dt.float32
F16 = mybir.dt.float16
NPDT = np.float16

LAST_RESULTS = None  # BassKernelResults of the most recent run (for test.py)


def _ensure_ntff_hook():
    """Provide antenv.axon_hooks if the image lacks it, so trace=True works."""
    try:
        from antenv.axon_hooks import get_axon_ntff_profile_hook  # noqa: F401
        return
    except ImportError:
        pass
    try:
        from trn_agent_boot.trn_boot import _ntff_profile_via_ctypes
        hook = _ntff_profile_via_ctypes("/opt/axon/libaxon_pjrt.so")
    except Exception:
        hook = None
    mod = types.ModuleType("antenv.axon_hooks")
    _state = {"hook": hook}
    mod.get_axon_ntff_profile_hook = lambda: _state["hook"]
    mod.set_axon_ntff_profile_hook = lambda h: _state.update(hook=h)
    sys.modules["antenv.axon_hooks"] = mod
    try:
        import antenv
        antenv.axon_hooks = mod
    except ImportError:
        pass


def _basis():
    t = np.arange(N)
    cols = [np.ones(N) / np.sqrt(N)]
    for k in range(1, KMAX):
        cols.append(np.sqrt(2.0 / N) * np.cos(2 * np.pi * k * t / N))
        cols.append(-np.sqrt(2.0 / N) * np.sin(2 * np.pi * k * t / N))
    return np.stack(cols, axis=1).astype(np.float32)  # [256, 31]


def _build_nc():
    nc = bacc.Bacc("TRN2", target_bir_lowering=False, debug=False,
                   enable_asserts=False, num_devices=8)

    xc = nc.declare_dram_parameter("xc", [M, FREE], F16, isOutput=False)
    w2 = nc.declare_dram_parameter("w2", [128, 64], F16, isOutput=False)
    wzl = nc.declare_dram_parameter("wzl", [32, CW], F16, isOutput=False)
    wzr = nc.declare_dram_parameter("wzr", [31, CW], F16, isOutput=False)
    out = nc.declare_dram_parameter("out", [M, FREE], F16, isOutput=True)

    with tile.TileContext(nc) as tc:
        with (
            tc.tile_pool(name="const", bufs=1) as constp,
            tc.tile_pool(name="xin", bufs=4) as xinp,
            tc.tile_pool(name="tt", bufs=4) as ttp,
            tc.tile_pool(name="oput", bufs=3) as outp,
            tc.tile_pool(name="pcd", bufs=2, space=bass.MemorySpace.PSUM) as pcdp,
            tc.tile_pool(name="p2", bufs=2, space=bass.MemorySpace.PSUM) as p2p,
        ):
            # consts ride SWDGE so the ACT ring is free for transposes
            w2sb = constp.tile([128, 64], F16)
            nc.gpsimd.dma_start(out=w2sb[:], in_=w2[:])
            # rings of L/R buffers; const rows filled once per buffer
            lgs, rgs = [], []
            for r_ in range(NLR):
                lg_t = constp.tile([64, CW], F16, name=f"lg{r_}")
                nc.gpsimd.dma_start(out=lg_t[0:32, :], in_=wzl[:])
                lgs.append(lg_t)
                rg_t = constp.tile([63, CW], F16, name=f"rg{r_}")
                nc.gpsimd.dma_start(out=rg_t[32:63, :], in_=wzr[:])
                rgs.append(rg_t)

            # Phase A: all input + transpose dispatches up front so no
            # per-chunk compute dependency ever blocks an engine's DMA queue.
            # Input of chunk g rides ring g%2, its transpose the other ring.
            # The XBAR is shared non-reentrant hardware: two DMA
            # transposes must NEVER be in flight concurrently, so they all
            # ride ONE ring (SP / Sync engine, which is otherwise idle).
            # Inputs ride the ACT ring.
            xins, tts = [], []
            for g in range(NCH):
                gsl = slice(g * CW, (g + 1) * CW)
                xin = xinp.tile([128, 2, CW], F16, tag="xin")
                nc.scalar.dma_start(
                    out=xin[:],
                    in_=xc[:, gsl].rearrange("(h p) c -> p h c", h=2))
                # XBAR transpose (SBUF->SBUF) of the whole chunk:
                # tt[n', gg, q] = xin[q, gg//8, (gg%8)*128 + n']
                # col (h*1024 + il*256 + n) -> gg = h*8 + 2*il + n//128
                tt = ttp.tile([128, 2 * CW // 128, 128], F16, tag="tt")
                nc.sync.dma_start(out=tt[:], in_=xin[:], transpose=True)
                xins.append(xin)
                tts.append(tt)

            # Phase B: compute per chunk
            outs_sp, outs_act = [], []
            for g in range(NCH):
                gsl = slice(g * CW, (g + 1) * CW)
                Lg = lgs[g % NLR]
                Rg = rgs[g % NLR]
                xin = xins[g]
                # view [128, kh, x, 128] with gg = x*2 + kh, x = h*4 + il
                tt_r = tts[g][:].rearrange("p (x y) m -> p y x m", y=2)

                pcd = pcdp.tile([64, 2 * 512], F32, tag="pcd")
                dv = pcd[32:64, :].rearrange("p (i m) -> p i m", i=CHC)
                for f in range(2):  # 512-col f-block = 2 channels
                    fsl = slice(f * 512, (f + 1) * 512)
                    # C = W^T X into rows 0..30 (31: zero col of w2)
                    nc.tensor.matmul(pcd[0:32, fsl], w2sb[:, 0:32],
                                     xin[:, 0, fsl], start=True, stop=False)
                    nc.tensor.matmul(pcd[0:32, fsl], w2sb[:, 32:64],
                                     xin[:, 1, fsl], start=False, stop=True)
                    # D = (X W)^T into rows 32..62 (63: zero); cols (il, h, m')
                    for h in range(2):
                        dout = dv[:, 2 * f:2 * f + 2, h * 128:(h + 1) * 128]
                        for kh in range(2):
                            x0 = h * 4 + 2 * f
                            nc.tensor.matmul(
                                dout, w2sb[:, kh * 32:(kh + 1) * 32],
                                tt_r[:, kh, x0:x0 + 2, :],
                                start=(kh == 0), stop=(kh == 1))
                # one wide copy per stream: D -> Lg (ACT), C -> Rg (DVE)
                nc.scalar.copy(Lg[32:64, :], pcd[32:64, :])
                nc.vector.tensor_copy(Rg[0:32, :], pcd[0:32, :])

                oo = outp.tile([128, 2, CW], F16, tag="oo")
                for j in range(2):  # m-half of the output
                    p2 = p2p.tile([128, CW], F32, tag="p2")
                    for ip in range(CHC):
                        c0 = ip * 256 + j * 128
                        nc.tensor.matmul(p2[:, ip * 256:(ip + 1) * 256],
                                         Lg[0:63, c0:c0 + 128],
                                         Rg[0:63, ip * 256:(ip + 1) * 256],
                                         start=True, stop=True)
                    nc.vector.tensor_copy(oo[:, j, :], p2[:])

                dst = out[:, gsl].rearrange("(j p) c -> p j c", j=2)
                if g >= NCH - 2:
                    # last chunks: SWDGE so they don't queue behind ring tails
                    nc.gpsimd.dma_start(out=dst, in_=oo[:])
                elif g >= NCH - 4:
                    outs_sp.append((dst, oo))
                else:
                    outs_act.append((dst, oo))

            # ring-tail outputs: FIFO-after all phase-A traffic.
            # Early chunks go on the ACT tail (free after inputs, ~13us);
            # later chunks on the SP tail (free after the transposes).
            for dst, oo in outs_act:
                nc.scalar.dma_start(out=dst, in_=oo[:])
            for dst, oo in outs_sp:
                nc.sync.dma_start(out=dst, in_=oo[:])

    nc.finalize()
    return nc


_NC = None


def kernel(x: np.ndarray) -> np.ndarray:
    global _NC, LAST_RESULTS
    x = np.asarray(x)
    assert x.shape == (B, M, N, I), x.shape

    W = _basis().astype(NPDT)          # [256, 31]
    Wt = W.T.copy()                    # [31, 256]
    zcol = np.zeros((128, 1), NPDT)
    w2_np = np.concatenate([W[0:128, :], zcol, W[128:256, :], zcol],
                           axis=1)                                # [128, 64]
    wtile = np.tile(Wt, (1, CHC))                                 # [31, CW]
    zrow = np.zeros((1, CW), NPDT)
    wzl_np = np.concatenate([wtile, zrow], axis=0)                # [32, CW]
    wzr_np = wtile                                                # [31, CW]

    if _NC is None:
        _NC = _build_nc()

    xq = np.asarray(x, dtype=NPDT)
    in_maps = []
    for b in range(B):
        xcm = np.ascontiguousarray(xq[b].transpose(0, 2, 1)).reshape(M, FREE)
        in_maps.append({
            "xc": xcm, "w2": w2_np, "wzl": wzl_np, "wzr": wzr_np,
        })

    trace = bool(int(os.environ.get("KERNEL_TRACE", "0")))
    if trace:
        _ensure_ntff_hook()
    last_err = None
    for attempt in range(3):
        try:
            LAST_RESULTS = run_bass_kernel_spmd(_NC, in_maps, list(range(B)),
                                                trace=trace and attempt == 0)
            break
        except Exception as e:  # rare transient NRT_EXEC_UNIT_UNRECOVERABLE
            last_err = e
            import time as _time
            _time.sleep(2.0)
            try:
                import jax
                jax.clear_caches()
                jax.extend.backend.clear_backends()
            except Exception:
                pass
    else:
        raise last_err

    out = np.empty((B, M, N, I), np.float32)
    for b in range(B):
        dev = LAST_RESULTS.results[b]["out"].astype(np.float32).reshape(M, I, N)
        out[b] = dev.transpose(0, 2, 1)
    return out
